# revision 13
# baseline (speedup 1.0000x reference)
"""MoE-routed autoencoder (4 experts, 1024->512->128->512->1024) on 8 TRN2 cores.

Strategy:
- Host: sort atoms by expert symbol, deal each expert's atoms evenly across the
  8 cores, pad per-(core,expert) groups to a common per-expert capacity so one
  SPMD program serves all cores. Only the routed expert runs per atom (4x less
  compute than dense dispatch).
- Device: activations live in transposed layout [feat, atoms] so every layer is
  out[M=feat_tile, N=atom_cols] = W[K,M].T @ act[K, N] on the tensor engine
  (f16 operands, fp32 PSUM accumulation). Bias+ReLU fuse into the PSUM->SBUF
  eviction on the scalar engine ([feat,1] per-partition bias).
- DMA: host pre-lays out X and W in device tile order so each tile is one DMA
  with multi-KB contiguous bursts; X-in on the sync HWDGE queue, Y-out on the
  scalar HWDGE queue, weights/bias on the gpsimd SWDGE queue.
- Host: scatter the routed outputs back to the original atom order.
"""

import math

import numpy as np

N_CORES = 8


def _round_fp32r(a: np.ndarray) -> np.ndarray:
    """Round-to-nearest-even fp32 -> fp32r (11-bit mantissa), bitwise."""
    u = np.ascontiguousarray(a, dtype=np.float32).view(np.uint32)
    lsb = (u >> 12) & np.uint32(1)
    r = (u + np.uint32(0x7FF) + lsb) & np.uint32(0xFFFFF000)
    return r.view(np.float32)


_PROGRAM_CACHE: dict = {}

# matmul operand precision: "f16" (10-bit mantissa, half the DMA/LDW cost)
# or "f32r" (11-bit mantissa, fp32-sized operands). PSUM accumulation is
# fp32 either way.
_MODE = "f16"

# test-harness knobs: when _TRACE is set, the SPMD launch requests an NTFF
# profile and the BassKernelResults lands in _LAST["res"].
_TRACE = False
_LAST: dict = {}


def _plan(dims, tiles):
    """Flat-buffer offsets for the tile-order X / Y layouts."""
    D_IN, H1, LAT, D_OUT, E, C_tot = dims
    KC1 = D_IN // 128
    MC4 = D_OUT // 128
    xoff, yoff, seq = 0, 0, []
    for e in range(E):
        off, nt, T = tiles[e]
        for t in range(nt):
            seq.append((e, t, off + t * T, T, xoff, yoff))
            xoff += 128 * KC1 * T
            yoff += 128 * MC4 * T
    return seq, xoff, yoff


def _build_program(dims, tiles, use_bias, n_bias_cols, mode):
    import concourse.bass as bass  # noqa: F401
    import concourse.tile as tile
    from concourse import bacc, mybir

    D_IN, H1, LAT, D_OUT, E, C_tot = dims
    f32 = mybir.dt.float32
    mdt = mybir.dt.float16 if mode == "f16" else mybir.dt.float32r
    RELU = mybir.ActivationFunctionType.Relu
    IDENT = mybir.ActivationFunctionType.Identity

    KC1 = D_IN // 128   # 8  k-chunks layer 1
    MC1 = H1 // 128     # 4  m-chunks layer 1
    KC2 = H1 // 128     # 4
    MC2 = LAT // 128    # 1
    KC3 = LAT // 128    # 1
    MC3 = H1 // 128     # 4
    KC4 = H1 // 128     # 4
    MC4 = D_OUT // 128  # 8

    seq, x_total, y_total = _plan(dims, tiles)

    nc = bacc.Bacc("TRN2", target_bir_lowering=False, debug=False,
                   num_devices=N_CORES)
    xt = nc.dram_tensor("xt", [x_total], mdt, kind="ExternalInput").ap()
    w1 = nc.dram_tensor("w1", [E, 128, KC1 * H1], mdt,
                        kind="ExternalInput").ap()
    w2 = nc.dram_tensor("w2", [E, 128, KC2 * LAT], mdt,
                        kind="ExternalInput").ap()
    w3 = nc.dram_tensor("w3", [E, 128, KC3 * H1], mdt,
                        kind="ExternalInput").ap()
    w4 = nc.dram_tensor("w4", [E, 128, KC4 * D_OUT], mdt,
                        kind="ExternalInput").ap()
    if use_bias:
        bias = nc.dram_tensor("bias", [128, n_bias_cols], f32,
                              kind="ExternalInput").ap()
    yt = nc.dram_tensor("yt", [y_total], f32, kind="ExternalOutput").ap()

    with tile.TileContext(nc) as tc:
        with (
            tc.tile_pool(name="wp", bufs=2) as wp,
            tc.tile_pool(name="xp", bufs=4) as xp,
            tc.tile_pool(name="hp", bufs=3) as hp,
            tc.tile_pool(name="zp", bufs=3) as zp,
            tc.tile_pool(name="dp", bufs=3) as dp,
            tc.tile_pool(name="yp", bufs=3) as yp,
            tc.tile_pool(name="bp", bufs=1) as bp,
            tc.tile_pool(name="ppa", bufs=4, space="PSUM") as ppa,
            tc.tile_pool(name="ppb", bufs=4, space="PSUM") as ppb,
        ):
            if use_bias:
                btile = bp.tile([128, n_bias_cols], f32)
                nc.gpsimd.dma_start(btile[:], bias[:])
                bias_col = [0]

                def next_bias():
                    c = bias_col[0]
                    bias_col[0] += 1
                    return btile[:, c:c + 1]

            def evict(out_ap, ps_ap, relu, on_vector=False):
                if relu:
                    b = next_bias() if use_bias else 0.0
                    nc.scalar.activation(out_ap, ps_ap, RELU, bias=b)
                elif use_bias:
                    nc.scalar.activation(out_ap, ps_ap, IDENT, bias=next_bias())
                elif on_vector:
                    nc.vector.tensor_copy(out_ap, ps_ap)
                else:
                    nc.scalar.activation(out_ap, ps_ap,
                                         mybir.ActivationFunctionType.Copy)

            wtiles = {}

            def load_weights(e):
                # expert 0's W1 rides the fast sync HWDGE queue so the PE can
                # start ~3us in; everything else streams on the gpsimd queue.
                dma0 = nc.sync.dma_start if e == 0 else nc.gpsimd.dma_start
                w1t = wp.tile([128, KC1 * H1], mdt, tag="w1")
                dma0(w1t[:], w1[e])
                w2t = wp.tile([128, KC2 * LAT], mdt, tag="w2")
                nc.gpsimd.dma_start(w2t[:], w2[e])
                w3t = wp.tile([128, KC3 * H1], mdt, tag="w3")
                nc.gpsimd.dma_start(w3t[:], w3[e])
                w4t = wp.tile([128, KC4 * D_OUT], mdt, tag="w4")
                nc.gpsimd.dma_start(w4t[:], w4[e])
                wtiles[e] = (w1t, w2t, w3t, w4t)

            load_weights(0)
            cur_e = 0
            for e, t, co, T, xo, yo in seq:
                if e != cur_e:
                    load_weights(e)
                    wtiles.pop(cur_e)
                    cur_e = e
                w1t, w2t, w3t, w4t = wtiles[e]

                xtile = xp.tile([128, KC1 * T], mdt, tag="x")
                nc.sync.dma_start(
                    xtile[:],
                    xt[xo:xo + 128 * KC1 * T].rearrange("(p f) -> p f", p=128))

                # L1: h[H1, T] = relu(W1.T @ x)
                htile = hp.tile([128, MC1 * T], mdt, tag="h")
                for m in range(MC1):
                    ps = ppa.tile([128, T], f32, tag="ps")
                    for k in range(KC1):
                        nc.tensor.matmul(
                            ps[:],
                            w1t[:, k * H1 + m * 128:k * H1 + (m + 1) * 128],
                            xtile[:, k * T:(k + 1) * T],
                            start=(k == 0), stop=(k == KC1 - 1))
                    evict(htile[:, m * T:(m + 1) * T], ps[:], relu=True)

                # L2: z[LAT, T] = relu(W2.T @ h)
                ztile = zp.tile([128, MC2 * T], mdt, tag="z")
                for m in range(MC2):
                    ps = ppa.tile([128, T], f32, tag="ps")
                    for k in range(KC2):
                        nc.tensor.matmul(
                            ps[:],
                            w2t[:, k * LAT + m * 128:k * LAT + (m + 1) * 128],
                            htile[:, k * T:(k + 1) * T],
                            start=(k == 0), stop=(k == KC2 - 1))
                    evict(ztile[:, m * T:(m + 1) * T], ps[:], relu=True)

                # L3: d[H1, T] = relu(W3.T @ z)
                dtile = dp.tile([128, MC3 * T], mdt, tag="d")
                for m in range(MC3):
                    ps = ppa.tile([128, T], f32, tag="ps")
                    for k in range(KC3):
                        nc.tensor.matmul(
                            ps[:],
                            w3t[:, k * H1 + m * 128:k * H1 + (m + 1) * 128],
                            ztile[:, k * T:(k + 1) * T],
                            start=(k == 0), stop=(k == KC3 - 1))
                    evict(dtile[:, m * T:(m + 1) * T], ps[:], relu=True)

                # L4: y[D_OUT, T] = W4.T @ d  (no relu)
                ytile = yp.tile([128, MC4 * T], f32, tag="y")
                for m in range(MC4):
                    ps = ppb.tile([128, T], f32, tag="ps")
                    for k in range(KC4):
                        nc.tensor.matmul(
                            ps[:],
                            w4t[:, k * D_OUT + m * 128:k * D_OUT + (m + 1) * 128],
                            dtile[:, k * T:(k + 1) * T],
                            start=(k == 0), stop=(k == KC4 - 1))
                    evict(ytile[:, m * T:(m + 1) * T], ps[:], relu=False,
                          on_vector=(m % 2 == 0))
                nc.sync.dma_start(
                    yt[yo:yo + 128 * MC4 * T].rearrange("(p f) -> p f", p=128),
                    ytile[:])

    nc.compile()
    return nc


def kernel(**inputs) -> np.ndarray:
    from concourse.bass_utils import run_bass_kernel_spmd

    X = np.ascontiguousarray(inputs["X"], dtype=np.float32)
    sym_ids = np.asarray(inputs["sym_ids"]).astype(np.int64).ravel()
    We = [inputs["We1"], inputs["We2"], inputs["Wd1"], inputs["Wd2"]]
    be = [np.asarray(inputs["be1"], dtype=np.float32),
          np.asarray(inputs["be2"], dtype=np.float32),
          np.asarray(inputs["bd1"], dtype=np.float32),
          np.asarray(inputs["bd2"], dtype=np.float32)]

    N, D_IN = X.shape
    E, _, H1 = We[0].shape
    LAT = We[1].shape[2]
    D_OUT = We[3].shape[2]
    KC1 = D_IN // 128
    MC4 = D_OUT // 128
    use_bias = any(np.any(b) for b in be)

    # ---- host routing: per-expert, per-core index assignment ----
    core_idx = [[None] * E for _ in range(N_CORES)]
    C_e = [0] * E
    for e in range(E):
        idx = np.flatnonzero(sym_ids == e)
        n = len(idx)
        base, rem = divmod(n, N_CORES)
        s = 0
        for c in range(N_CORES):
            cnt = base + (1 if c < rem else 0)
            core_idx[c][e] = idx[s:s + cnt]
            s += cnt
        C_e[e] = base + (1 if rem else 0)

    # per-expert column tiling: n_t tiles of width T (multiple of 8, <=512)
    tiles = []
    off = 0
    for e in range(E):
        ce = max(C_e[e], 1)
        nt = max(1, math.ceil(ce / 512))
        T = -(-math.ceil(ce / nt) // 8) * 8
        tiles.append((off, nt, T))
        off += nt * T
    C_tot = off

    # ---- build / fetch compiled program ----
    dims = (D_IN, H1, LAT, D_OUT, E, C_tot)
    n_bias_cols = E * (H1 + LAT + H1 + D_OUT) // 128
    key = (dims, tuple(tiles), use_bias, _MODE)
    nc = _PROGRAM_CACHE.get(key)
    if nc is None:
        nc = _build_program(dims, tiles, use_bias, n_bias_cols, _MODE)
        _PROGRAM_CACHE[key] = nc

    # ---- prepare inputs ----
    if _MODE == "f16":
        rnd = lambda a: np.ascontiguousarray(a, dtype=np.float32).astype(
            np.float16)
        mm_np = np.float16
    else:
        rnd = _round_fp32r
        mm_np = np.float32
    XrT = np.ascontiguousarray(rnd(X).T)                     # [D_IN, N]
    XrT_z = np.concatenate(
        [XrT, np.zeros((D_IN, 1), mm_np)], axis=1)           # pad col = N

    # weights in device layout: [E, 128, kc*m] (k-chunk-major columns)
    def wdev(w, kc, mw):
        return np.ascontiguousarray(
            rnd(w).reshape(E, kc, 128, mw).transpose(0, 2, 1, 3)
            .reshape(E, 128, kc * mw))

    Wd = [wdev(We[0], KC1, H1), wdev(We[1], H1 // 128, LAT),
          wdev(We[2], LAT // 128, H1), wdev(We[3], H1 // 128, D_OUT)]

    seq, x_total, y_total = _plan(dims, tiles)

    bias_h = None
    if use_bias:
        bias_h = np.zeros((128, n_bias_cols), np.float32)
        col = 0
        for e in range(E):
            for b in (be[0][e], be[1][e], be[2][e], be[3][e]):
                for mch in range(len(b) // 128):
                    bias_h[:, col] = b[mch * 128:(mch + 1) * 128]
                    col += 1

    perms = []
    in_maps = []
    for c in range(N_CORES):
        perm = np.full(C_tot, N, dtype=np.int64)
        for e in range(E):
            o = tiles[e][0]
            idx = core_idx[c][e]
            perm[o:o + len(idx)] = idx
        perms.append(perm)
        g3 = XrT_z[:, perm].reshape(KC1, 128, C_tot)
        xflat = np.empty(x_total, dtype=mm_np)
        for e, t, co, T, xo, yo in seq:
            xflat[xo:xo + 128 * KC1 * T] = (
                g3[:, :, co:co + T].transpose(1, 0, 2).reshape(-1))
        m = {"xt": xflat, "w1": Wd[0], "w2": Wd[1], "w3": Wd[2], "w4": Wd[3]}
        if use_bias:
            m["bias"] = bias_h
        in_maps.append(m)

    res = run_bass_kernel_spmd(nc, in_maps, core_ids=list(range(N_CORES)),
                               trace=_TRACE)
    _LAST["res"] = res

    # ---- unshard ----
    Y = np.empty((N, D_OUT), dtype=np.float32)
    for c in range(N_CORES):
        yflat = res.results[c]["yt"]
        ytc = np.empty((D_OUT, C_tot), dtype=np.float32)
        for e, t, co, T, xo, yo in seq:
            ytc[:, co:co + T] = (
                yflat[yo:yo + 128 * MC4 * T].reshape(128, MC4, T)
                .transpose(1, 0, 2).reshape(D_OUT, T))
        perm = perms[c]
        valid = perm != N
        Y[perm[valid]] = ytc.T[valid]
    return Y
